# revision 13
# baseline (speedup 1.0000x reference)
"""Multi-head causal attention on 8 Trainium2 NeuronCores.

Problem: X [2, 2048, 1024] f32, W_q/W_k/W_v [1024, 1024], W_o [1024, 1024],
b_o [1024]; 16 heads, head_dim 64, causal softmax attention + out projection.

Sharding: 2 (batch) x 4 (head-blocks of 4 heads) = 8 cores. Each core
computes q/k/v for its 4 heads on its batch, causal attention, and a partial
output projection ctx @ W_o[rows]. Host sums the 4 partials per batch and
adds b_o. No cross-core collectives.

v2 vs v1 (233us):
  - X transposed by DMA (XBAR dma_start_transpose) instead of PE matmuls.
  - Single pool scope: QKV projection work is emitted as "filler" inside the
    attention k-block loop so the Tile scheduler overlaps it with the
    ACT-bound softmax-exp stream (exp = (N+352)/1.2 ns on ScalarE is the
    phase-B floor).
  - ScalarE kept exp-only (all copies pinned to nc.vector; ones-columns via
    gpsimd memset).
  - Normalization: reciprocal_approx_fast (1 DVE op, ~5x faster than
    reciprocal) + gpsimd partition_broadcast instead of DVE reciprocal +
    DMA round-trip broadcasts; ctx drained unnormalized to bf16 and scaled
    in place.
  - PSUM: s x2 (4 banks) + ctx1 + ctx2 + qkv + oproj = 8 banks.
  - exp always one [128,1024] call per k-block (scale 0.125 folded into W_q
    host-side); AV skips fully-masked columns as before.
"""
import sys

sys.path.insert(0, "/opt/trn_rl_repo")

import numpy as np

NEG = -1.0e9
B, NTOK, DIN = 2, 2048, 1024
NH, HD = 16, 64
HPC = 4            # heads per core
CLOC = HPC * HD    # 256 local channels
NCORES = 8
NTB = NTOK // 128  # 16 token blocks
NQC = NTOK // 512  # 4 q-chunks
NCB = DIN // 128   # 8 contraction blocks

_CACHE = {}
_last_in_maps = None


def _build():
    from concourse import bacc, mybir, tile

    F32 = mybir.dt.float32
    BF16 = mybir.dt.bfloat16
    EXP = mybir.ActivationFunctionType.Exp
    P = 128

    nc = bacc.Bacc(None)
    Xl = nc.declare_dram_parameter("Xl", [NTOK, DIN], BF16, isOutput=False)
    Wq = nc.declare_dram_parameter("Wq", [DIN, CLOC], BF16, isOutput=False)
    Wk = nc.declare_dram_parameter("Wk", [DIN, CLOC], BF16, isOutput=False)
    Wv = nc.declare_dram_parameter("Wv", [DIN, CLOC], BF16, isOutput=False)
    Wo = nc.declare_dram_parameter("Wo", [CLOC, DIN], BF16, isOutput=False)
    tri = nc.declare_dram_parameter("tri", [P, P], F32, isOutput=False)
    out = nc.declare_dram_parameter("out", [NTOK, DIN], F32, isOutput=True)

    with tile.TileContext(nc) as tc:
        with (
            tc.tile_pool(name="const", bufs=1) as constp,
            tc.tile_pool(name="xt", bufs=1) as xtp,
            tc.tile_pool(name="w", bufs=1) as wp,
            tc.tile_pool(name="qkT", bufs=1) as qkTp,
            tc.tile_pool(name="vt", bufs=1) as vtp,
            tc.tile_pool(name="ctxn", bufs=1) as ctxnp,
            tc.tile_pool(name="att", bufs=1) as attp,
            tc.tile_pool(name="osb", bufs=1) as osbp,
            tc.tile_pool(name="dsc", bufs=1, space="DRAM") as dscp,
            tc.tile_pool(name="ps", bufs=1, space="PSUM") as psp,
        ):
            tri_sb = constp.tile([P, P], F32, tag="tri")
            nc.sync.dma_start(tri_sb[:], tri[:])

            # weights
            wq_sb = [wp.tile([P, CLOC], BF16, tag=f"wq{cb}", name=f"wq{cb}")
                     for cb in range(NCB)]
            wk_sb = [wp.tile([P, CLOC], BF16, tag=f"wk{cb}", name=f"wk{cb}")
                     for cb in range(NCB)]
            wv_sb = [wp.tile([P, CLOC], BF16, tag=f"wv{cb}", name=f"wv{cb}")
                     for cb in range(NCB)]
            wo_sb = [wp.tile([P, DIN], BF16, tag=f"wo{p}", name=f"wo{p}")
                     for p in range(2)]
            for cb in range(NCB):
                sl = slice(cb * P, (cb + 1) * P)
                nc.sync.dma_start(wq_sb[cb][:], Wq[sl, :])
                nc.sync.dma_start(wk_sb[cb][:], Wk[sl, :])
                nc.sync.dma_start(wv_sb[cb][:], Wv[sl, :])
            for p in range(2):
                nc.sync.dma_start(wo_sb[p][:], Wo[p * P:(p + 1) * P, :])

            # X transposed straight from DRAM: XT[cb][qc] [128, 512]
            XT = [
                [xtp.tile([P, 512], BF16, tag=f"xt{cb}_{q}", name=f"xt{cb}_{q}")
                 for q in range(NQC)]
                for cb in range(NCB)
            ]
            for q in range(NQC):
                for cb in range(NCB):
                    nc.scalar.dma_start_transpose(
                        XT[cb][q][:],
                        Xl[q * 512:(q + 1) * 512, cb * P:(cb + 1) * P],
                    )

            # static result tiles
            qT = [qkTp.tile([P, NTOK], BF16, tag=f"qT{p}", name=f"qT{p}")
                  for p in range(2)]
            kT = [qkTp.tile([P, NTOK], BF16, tag=f"kT{p}", name=f"kT{p}")
                  for p in range(2)]
            # v tiles [128 keys, 192]: [vA | ones | pad | ones | pad | vB]
            vt = [
                [vtp.tile([P, 192], BF16, tag=f"vt{p}_{tb}", name=f"vt{p}_{tb}")
                 for tb in range(NTB)]
                for p in range(2)
            ]
            ctxn = [
                [ctxnp.tile([P, 512], BF16, tag=f"ctxn{p}_{qc}",
                            name=f"ctxn{p}_{qc}") for qc in range(NQC)]
                for p in range(2)
            ]
            for p in range(2):
                for tb in range(NTB):
                    t = vt[p][tb]
                    nc.gpsimd.memset(t[:, 64:128], 0.0)
                    nc.gpsimd.memset(t[:, 64:65], 1.0)
                    nc.gpsimd.memset(t[:, 96:97], 1.0)

            # ---------------- emission helpers ----------------
            def emit_qk_half(w_sb, dst, p, qc, half, state):
                if half == 0:
                    state["t"] = psp.tile([P, 512], F32, tag="qkv", bufs=1,
                                          name="qk_ps")
                t = state["t"]
                csl = slice(p * P, (p + 1) * P)
                for cb in range(4 * half, 4 * half + 4):
                    nc.tensor.matmul(
                        t[:], w_sb[cb][:, csl], XT[cb][qc][:],
                        start=(cb == 0), stop=(cb == NCB - 1),
                    )
                if half == 1:
                    nc.vector.tensor_copy(
                        dst[p][:, qc * 512:(qc + 1) * 512], t[:]
                    )

            def emit_v(tb):
                t = psp.tile([P, 512], F32, tag="qkv", bufs=1, name="v_ps")
                for cb in range(NCB):
                    nc.tensor.matmul(
                        t[:, 0:CLOC],
                        XT[cb][tb // 4][:, (tb % 4) * P:(tb % 4 + 1) * P],
                        wv_sb[cb][:],
                        start=(cb == 0), stop=(cb == NCB - 1),
                    )
                for p in range(2):
                    hA, hB = 2 * p, 2 * p + 1
                    nc.vector.tensor_copy(
                        vt[p][tb][:, 0:64], t[:, hA * 64:(hA + 1) * 64]
                    )
                    nc.vector.tensor_copy(
                        vt[p][tb][:, 128:192], t[:, hB * 64:(hB + 1) * 64]
                    )

            def qkv_closures(qc):
                cl = []
                for tb in range(4 * qc, 4 * qc + 4):
                    cl.append((("v", tb), lambda tb=tb: emit_v(tb)))
                for p in range(2):
                    for nm, w_sb, dst in (("q", wq_sb, qT), ("k", wk_sb, kT)):
                        st = {}
                        for half in range(2):
                            cl.append((
                                (nm, p, qc),
                                lambda w_sb=w_sb, dst=dst, p=p, half=half,
                                st=st: emit_qk_half(w_sb, dst, p, qc, half, st)
                            ))
                return cl

            def emit_outproj_tile(qc, i, jc):
                tb = 4 * qc + i
                tsl = slice(tb * P, (tb + 1) * P)
                jsl = slice(jc * 512, (jc + 1) * 512)
                o_ps = psp.tile([P, 512], F32, tag="oproj", bufs=1,
                                name="o_ps")
                for pp in range(2):
                    nc.tensor.matmul(
                        o_ps[:],
                        ctxn[pp][qc][:, i * P:(i + 1) * P],
                        wo_sb[pp][:, jsl],
                        start=(pp == 0), stop=(pp == 1),
                    )
                o_sb = osbp.tile([P, 512], F32, tag="o_sb", bufs=4,
                                 name="o_sb")
                nc.vector.tensor_copy(o_sb[:], o_ps[:])
                nc.sync.dma_start(out[tsl, jsl], o_sb[:])

            def outproj_closures(qc):
                return [
                    (("o", qc, i, jc),
                     lambda qc=qc, i=i, jc=jc: emit_outproj_tile(qc, i, jc))
                    for i in range(4) for jc in range(2)
                ]

            # ---------------- attention ----------------
            filler = []       # (key, closure) of ~<=1us of PE work each
            pending = []      # deferred normalization closures

            def flush_pending():
                for fn in pending:
                    fn()
                pending.clear()

            def pop_filler():
                if filler:
                    filler.pop(0)[1]()

            def drain_until(keys):
                # emit queued closures until all `keys` have fully emitted
                need = {}
                for k in keys:
                    n = sum(1 for e in filler if e[0] == k)
                    if n:
                        need[k] = n
                while need:
                    k, fn = filler.pop(0)
                    fn()
                    if k in need:
                        need[k] -= 1
                        if need[k] == 0:
                            del need[k]

            # qkv for qc=0 emitted up front; q/k for (p=1, qc=0) via filler
            for tb in range(4):
                emit_v(tb)
            for nm, w_sb, dst in (("q", wq_sb, qT), ("k", wk_sb, kT)):
                st = {}
                for half in range(2):
                    emit_qk_half(w_sb, dst, 0, 0, half, st)
            for nm, w_sb, dst in (("q", wq_sb, qT), ("k", wk_sb, kT)):
                st = {}
                for half in range(2):
                    filler.append((
                        (nm, 1, 0),
                        lambda w_sb=w_sb, dst=dst, half=half, st=st:
                        emit_qk_half(w_sb, dst, 1, 0, half, st)
                    ))

            prev = [None]          # pipelined AV across kb boundaries
            chunks_flushed = [-1]  # highest chunk idx whose norm is emitted
            oproj_pushed = [0]     # next qc whose outproj closures to push

            for qc in range(NQC):
                if qc + 1 < NQC:
                    filler.extend(qkv_closures(qc + 1))
                for p in range(2):
                    # this chunk's q/k/v producers must be emitted (priority-
                    # ordered) before its first scores matmul
                    drain_until(
                        [("q", p, qc), ("k", p, qc)]
                        + [("v", tb) for tb in range(4 * qc + 4)]
                    )
                    qsl = slice(qc * 512, (qc + 1) * 512)
                    nkb = 4 * qc + 4
                    ctx1 = psp.tile([P, 512], F32, tag="ctx1", bufs=1)
                    ctx2 = psp.tile([P, 512], F32, tag="ctx2", bufs=1)

                    def av(kb, expT, nkb=nkb, p=p, qc=qc,
                           ctx1=ctx1, ctx2=ctx2):
                        st, sp = kb == 0, kb == nkb - 1
                        oi = kb - 4 * qc
                        off = 128 * oi if oi > 0 else 0
                        nc.tensor.matmul(
                            ctx1[:, off:512], vt[p][kb][:, 0:128],
                            expT[:, off:512],
                            start=st, stop=sp, skip_group_check=(off > 0),
                        )
                        nc.tensor.matmul(
                            ctx2[:, off:512], vt[p][kb][:, 64:192],
                            expT[:, 512 + off:1024],
                            start=st, stop=sp, skip_group_check=(off > 0),
                        )
                        if not sp:
                            return
                        # chunk close: den rows + unnormalized ctx -> bf16
                        den = attp.tile([P, 512], F32, tag="den", bufs=2,
                                        name="den")
                        nc.vector.tensor_copy(den[64:65, :], ctx1[64:65, :])
                        nc.vector.tensor_copy(den[32:33, :], ctx2[32:33, :])
                        nc.vector.tensor_copy(
                            ctxn[p][qc][0:64, :], ctx1[0:64, :]
                        )
                        nc.vector.tensor_copy(
                            ctxn[p][qc][64:128, :], ctx2[64:128, :]
                        )

                        def norm(den=den, p=p, qc=qc):
                            rec = attp.tile([P, 512], F32, tag="rec",
                                            bufs=2, name="rec")
                            nc.vector.reciprocal(rec[:, :], den[:, :])
                            d_t = dscp.tile([2, 512], F32, tag="d", bufs=2,
                                            name="d_t")
                            nc.sync.dma_start(d_t[0:1, :], rec[64:65, :])
                            nc.sync.dma_start(d_t[1:2, :], rec[32:33, :])
                            bc = attp.tile([P, 512], F32, tag="bc",
                                           bufs=2, name="bc")
                            nc.sync.dma_start(
                                bc[0:64, :],
                                d_t[0:1, :].to_broadcast((64, 512)),
                            )
                            nc.sync.dma_start(
                                bc[64:128, :],
                                d_t[1:2, :].to_broadcast((64, 512)),
                            )
                            nc.vector.tensor_mul(
                                ctxn[p][qc][0:64, :], ctxn[p][qc][0:64, :],
                                bc[0:64, :],
                            )
                            nc.vector.tensor_mul(
                                ctxn[p][qc][64:128, :],
                                ctxn[p][qc][64:128, :], bc[64:128, :],
                            )

                        pending.append(norm)

                    for kb in range(nkb):
                        if kb == 4:
                            flush_pending()
                            chunks_flushed[0] = 2 * qc + p - 1
                            while (oproj_pushed[0] + 1) * 2 - 1 <= chunks_flushed[0]:
                                filler.extend(outproj_closures(oproj_pushed[0]))
                                oproj_pushed[0] += 1
                        pop_filler()
                        ksl = slice(kb * P, (kb + 1) * P)
                        s_ps = psp.tile([P, 1024], F32, tag="s", bufs=2)
                        nc.tensor.matmul(
                            s_ps[:, 0:512], kT[p][0:64, ksl], qT[p][0:64, qsl],
                            start=True, stop=True, tile_position=(0, 0),
                        )
                        nc.tensor.matmul(
                            s_ps[:, 512:1024], kT[p][64:128, ksl],
                            qT[p][64:128, qsl],
                            start=True, stop=True, tile_position=(64, 0),
                        )
                        oi = kb - 4 * qc
                        off = 128 * oi
                        if oi >= 0:
                            # triangular strip mask on both halves
                            nc.vector.tensor_add(
                                s_ps[:, off:off + 128], s_ps[:, off:off + 128],
                                tri_sb[:],
                            )
                            nc.vector.tensor_add(
                                s_ps[:, 512 + off:640 + off],
                                s_ps[:, 512 + off:640 + off], tri_sb[:],
                            )
                        expT = attp.tile([P, 1024], BF16, tag="exp", bufs=6)
                        nc.scalar.activation(expT[:], s_ps[:], EXP)
                        if prev[0] is not None:
                            prev[0][0](*prev[0][1])
                        prev[0] = (av, (kb, expT))
            if prev[0] is not None:
                prev[0][0](*prev[0][1])
            flush_pending()
            while filler:
                filler.pop(0)[1]()
            while oproj_pushed[0] < NQC:
                for _, fn in outproj_closures(oproj_pushed[0]):
                    fn()
                oproj_pushed[0] += 1

    nc.compile()
    return nc


def _get_nc():
    if "nc" not in _CACHE:
        _CACHE["nc"] = _build()
    return _CACHE["nc"]


def kernel(X, W_q, W_k, W_v, W_o, b_o):
    import ml_dtypes
    from concourse.bass_utils import run_bass_kernel_spmd

    BF = ml_dtypes.bfloat16
    X = np.asarray(X, dtype=np.float32)
    # fold the 1/sqrt(head_dim) softmax scale into W_q
    W_q = (np.asarray(W_q, dtype=np.float32) * 0.125).astype(BF)
    W_k = np.asarray(W_k, dtype=np.float32).astype(BF)
    W_v = np.asarray(W_v, dtype=np.float32).astype(BF)
    W_o = np.asarray(W_o, dtype=np.float32).astype(BF)
    b_o = np.asarray(b_o, dtype=np.float32)
    Xb = X.astype(BF)

    nc = _get_nc()
    # triangular strip mask: row kp masks columns j < kp (key > query)
    kp = np.arange(128)[:, None]
    j = np.arange(128)[None, :]
    tri = np.where(kp <= j, 0.0, NEG).astype(np.float32)

    in_maps = []
    for c in range(NCORES):
        b = c // 4
        hb = c % 4
        cs = slice(hb * CLOC, (hb + 1) * CLOC)
        in_maps.append({
            "Xl": np.ascontiguousarray(Xb[b]),
            "Wq": np.ascontiguousarray(W_q[:, cs]),
            "Wk": np.ascontiguousarray(W_k[:, cs]),
            "Wv": np.ascontiguousarray(W_v[:, cs]),
            "Wo": np.ascontiguousarray(W_o[cs, :]),
            "tri": tri,
        })

    global _last_in_maps
    _last_in_maps = in_maps
    res = run_bass_kernel_spmd(nc, in_maps, list(range(NCORES)))
    out = np.empty((B, NTOK, DIN), dtype=np.float32)
    for b in range(B):
        acc = res.results[4 * b]["out"].astype(np.float32)
        for hb in range(1, 4):
            acc = acc + res.results[4 * b + hb]["out"]
        out[b] = acc + b_o[None, :]
    return out


# revision 16
# speedup vs baseline: 1.4612x; 1.4612x over previous
"""Multi-head causal attention on 8 Trainium2 NeuronCores.

Problem: X [2, 2048, 1024] f32, W_q/W_k/W_v [1024, 1024], W_o [1024, 1024],
b_o [1024]; 16 heads, head_dim 64, causal softmax attention + out projection.

Sharding: 2 (batch) x 4 (head-blocks of 4 heads) = 8 cores. Each core
computes q/k/v for its 4 heads on its batch, causal attention, and a partial
output projection ctx @ W_o[rows]. Host sums the 4 partials per batch and
adds b_o. No cross-core collectives.

v2 vs v1 (233us):
  - X transposed by DMA (XBAR dma_start_transpose) instead of PE matmuls.
  - Single pool scope: QKV projection work is emitted as "filler" inside the
    attention k-block loop so the Tile scheduler overlaps it with the
    ACT-bound softmax-exp stream (exp = (N+352)/1.2 ns on ScalarE is the
    phase-B floor).
  - ScalarE kept exp-only (all copies pinned to nc.vector; ones-columns via
    gpsimd memset).
  - Normalization: reciprocal_approx_fast (1 DVE op, ~5x faster than
    reciprocal) + gpsimd partition_broadcast instead of DVE reciprocal +
    DMA round-trip broadcasts; ctx drained unnormalized to bf16 and scaled
    in place.
  - PSUM: s x2 (4 banks) + ctx1 + ctx2 + qkv + oproj = 8 banks.
  - exp always one [128,1024] call per k-block (scale 0.125 folded into W_q
    host-side); AV skips fully-masked columns as before.
"""
import sys

sys.path.insert(0, "/opt/trn_rl_repo")

import numpy as np

NEG = -1.0e9
B, NTOK, DIN = 2, 2048, 1024
NH, HD = 16, 64
HPC = 4            # heads per core
CLOC = HPC * HD    # 256 local channels
NCORES = 8
NTB = NTOK // 128  # 16 token blocks
NQC = NTOK // 512  # 4 q-chunks
NCB = DIN // 128   # 8 contraction blocks

_CACHE = {}
_last_in_maps = None


def _build():
    from concourse import bacc, mybir, tile

    F32 = mybir.dt.float32
    BF16 = mybir.dt.bfloat16
    EXP = mybir.ActivationFunctionType.Exp
    P = 128

    nc = bacc.Bacc(None)
    # host-transposed X in (qc, cb)-block-contiguous layout: block (qc, cb)
    # at rows [(qc*NCB+cb)*128, +128) is XT[cb*128:(cb+1)*128, qc*512:+512]
    Xr = nc.declare_dram_parameter("Xr", [NQC * NCB * P, 512], BF16,
                                   isOutput=False)
    # weights cb-major along columns: [:, cb*256:(cb+1)*256] = W[cb block]
    Wq = nc.declare_dram_parameter("Wq", [P, NCB * CLOC], BF16, isOutput=False)
    Wk = nc.declare_dram_parameter("Wk", [P, NCB * CLOC], BF16, isOutput=False)
    Wv = nc.declare_dram_parameter("Wv", [P, NCB * CLOC], BF16, isOutput=False)
    Wo = nc.declare_dram_parameter("Wo", [CLOC, DIN], BF16, isOutput=False)
    tri = nc.declare_dram_parameter("tri", [P, P], F32, isOutput=False)
    out = nc.declare_dram_parameter("out", [NTOK, DIN], F32, isOutput=True)

    with tile.TileContext(nc) as tc:
        with (
            tc.tile_pool(name="const", bufs=1) as constp,
            tc.tile_pool(name="xt", bufs=1) as xtp,
            tc.tile_pool(name="w", bufs=1) as wp,
            tc.tile_pool(name="qkT", bufs=1) as qkTp,
            tc.tile_pool(name="vt", bufs=1) as vtp,
            tc.tile_pool(name="ctxn", bufs=1) as ctxnp,
            tc.tile_pool(name="att", bufs=1) as attp,
            tc.tile_pool(name="osb", bufs=1) as osbp,
            tc.tile_pool(name="dsc", bufs=1, space="DRAM") as dscp,
            tc.tile_pool(name="ps", bufs=1, space="PSUM") as psp,
        ):
            tri_sb = constp.tile([P, P], F32, tag="tri")
            nc.sync.dma_start(tri_sb[:], tri[:])

            # weights: one wide tile per matrix, single contiguous DMA each
            wq_sb = wp.tile([P, NCB * CLOC], BF16, tag="wq", name="wq")
            wk_sb = wp.tile([P, NCB * CLOC], BF16, tag="wk", name="wk")
            wv_sb = wp.tile([P, NCB * CLOC], BF16, tag="wv", name="wv")
            wo_sb = [wp.tile([P, DIN], BF16, tag=f"wo{p}", name=f"wo{p}")
                     for p in range(2)]
            XT = [
                [xtp.tile([P, 512], BF16, tag=f"xt{cb}_{q}", name=f"xt{cb}_{q}")
                 for q in range(NQC)]
                for cb in range(NCB)
            ]

            def load_xt(q):
                for cb in range(NCB):
                    r0 = (q * NCB + cb) * P
                    nc.sync.dma_start(XT[cb][q][:], Xr[r0:r0 + P, :])

            # ordered by first need; all on the sync queue (scalar=exp only)
            nc.sync.dma_start(wq_sb[:], Wq[:])
            load_xt(0)
            nc.sync.dma_start(wk_sb[:], Wk[:])
            nc.sync.dma_start(wv_sb[:], Wv[:])
            load_xt(1)
            for p in range(2):
                nc.sync.dma_start(wo_sb[p][:], Wo[p * P:(p + 1) * P, :])
            load_xt(2)
            load_xt(3)

            # static result tiles
            qT = [qkTp.tile([P, NTOK], BF16, tag=f"qT{p}", name=f"qT{p}")
                  for p in range(2)]
            kT = [qkTp.tile([P, NTOK], BF16, tag=f"kT{p}", name=f"kT{p}")
                  for p in range(2)]
            # v tiles [128 keys, 192]: [vA | ones | pad | ones | pad | vB]
            vt = [
                [vtp.tile([P, 192], BF16, tag=f"vt{p}_{tb}", name=f"vt{p}_{tb}")
                 for tb in range(NTB)]
                for p in range(2)
            ]
            ctxn = [
                [ctxnp.tile([P, 512], BF16, tag=f"ctxn{p}_{qc}",
                            name=f"ctxn{p}_{qc}") for qc in range(NQC)]
                for p in range(2)
            ]
            for p in range(2):
                for tb in range(NTB):
                    t = vt[p][tb]
                    nc.gpsimd.memset(t[:, 64:128], 0.0)
                    nc.gpsimd.memset(t[:, 64:65], 1.0)
                    nc.gpsimd.memset(t[:, 96:97], 1.0)

            # ---------------- emission helpers ----------------
            def emit_qk_half(w_sb, dst, p, qc, half, state):
                if half == 0:
                    state["t"] = psp.tile([P, 512], F32, tag="qkv", bufs=1,
                                          name="qk_ps")
                t = state["t"]
                for cb in range(4 * half, 4 * half + 4):
                    csl = slice(cb * CLOC + p * P, cb * CLOC + (p + 1) * P)
                    nc.tensor.matmul(
                        t[:], w_sb[:, csl], XT[cb][qc][:],
                        start=(cb == 0), stop=(cb == NCB - 1),
                    )
                if half == 1:
                    nc.vector.tensor_copy(
                        dst[p][:, qc * 512:(qc + 1) * 512], t[:]
                    )

            def emit_v(tb):
                t = psp.tile([P, 512], F32, tag="qkv", bufs=1, name="v_ps")
                for cb in range(NCB):
                    nc.tensor.matmul(
                        t[:, 0:CLOC],
                        XT[cb][tb // 4][:, (tb % 4) * P:(tb % 4 + 1) * P],
                        wv_sb[:, cb * CLOC:(cb + 1) * CLOC],
                        start=(cb == 0), stop=(cb == NCB - 1),
                    )
                for p in range(2):
                    hA, hB = 2 * p, 2 * p + 1
                    nc.vector.tensor_copy(
                        vt[p][tb][:, 0:64], t[:, hA * 64:(hA + 1) * 64]
                    )
                    nc.vector.tensor_copy(
                        vt[p][tb][:, 128:192], t[:, hB * 64:(hB + 1) * 64]
                    )

            def qkv_closures(qc):
                cl = []
                for tb in range(4 * qc, 4 * qc + 4):
                    cl.append((("v", tb), lambda tb=tb: emit_v(tb)))
                for p in range(2):
                    for nm, w_sb, dst in (("q", wq_sb, qT), ("k", wk_sb, kT)):
                        st = {}
                        for half in range(2):
                            cl.append((
                                (nm, p, qc),
                                lambda w_sb=w_sb, dst=dst, p=p, half=half,
                                st=st: emit_qk_half(w_sb, dst, p, qc, half, st)
                            ))
                return cl

            def emit_outproj_tile(qc, i, jc):
                tb = 4 * qc + i
                tsl = slice(tb * P, (tb + 1) * P)
                jsl = slice(jc * 512, (jc + 1) * 512)
                o_ps = psp.tile([P, 512], F32, tag="oproj", bufs=1,
                                name="o_ps")
                for pp in range(2):
                    nc.tensor.matmul(
                        o_ps[:],
                        ctxn[pp][qc][:, i * P:(i + 1) * P],
                        wo_sb[pp][:, jsl],
                        start=(pp == 0), stop=(pp == 1),
                    )
                o_sb = osbp.tile([P, 512], F32, tag="o_sb", bufs=4,
                                 name="o_sb")
                nc.vector.tensor_copy(o_sb[:], o_ps[:])
                nc.sync.dma_start(out[tsl, jsl], o_sb[:])

            def outproj_closures(qc):
                return [
                    (("o", qc, i, jc),
                     lambda qc=qc, i=i, jc=jc: emit_outproj_tile(qc, i, jc))
                    for i in range(4) for jc in range(2)
                ]

            # ---------------- attention ----------------
            filler = []       # (key, closure) of ~<=1us of PE work each
            pending = []      # deferred normalization closures

            def flush_pending():
                for fn in pending:
                    fn()
                pending.clear()

            def pop_filler():
                if filler:
                    filler.pop(0)[1]()

            def drain_until(keys):
                # emit queued closures until all `keys` have fully emitted
                need = {}
                for k in keys:
                    n = sum(1 for e in filler if e[0] == k)
                    if n:
                        need[k] = n
                while need:
                    k, fn = filler.pop(0)
                    fn()
                    if k in need:
                        need[k] -= 1
                        if need[k] == 0:
                            del need[k]

            # qkv for qc=0 emitted up front; q/k for (p=1, qc=0) via filler
            for tb in range(4):
                emit_v(tb)
            for nm, w_sb, dst in (("q", wq_sb, qT), ("k", wk_sb, kT)):
                st = {}
                for half in range(2):
                    emit_qk_half(w_sb, dst, 0, 0, half, st)
            for nm, w_sb, dst in (("q", wq_sb, qT), ("k", wk_sb, kT)):
                st = {}
                for half in range(2):
                    filler.append((
                        (nm, 1, 0),
                        lambda w_sb=w_sb, dst=dst, half=half, st=st:
                        emit_qk_half(w_sb, dst, 1, 0, half, st)
                    ))

            prev = [None]          # pipelined AV across kb boundaries
            chunks_flushed = [-1]  # highest chunk idx whose norm is emitted
            oproj_pushed = [0]     # next qc whose outproj closures to push

            for qc in range(NQC):
                if qc + 1 < NQC:
                    filler.extend(qkv_closures(qc + 1))
                for p in range(2):
                    # this chunk's q/k/v producers must be emitted (priority-
                    # ordered) before its first scores matmul
                    drain_until(
                        [("q", p, qc), ("k", p, qc)]
                        + [("v", tb) for tb in range(4 * qc + 4)]
                    )
                    qsl = slice(qc * 512, (qc + 1) * 512)
                    nkb = 4 * qc + 4
                    ctx1 = psp.tile([P, 512], F32, tag="ctx1", bufs=1)
                    ctx2 = psp.tile([P, 512], F32, tag="ctx2", bufs=1)

                    def av(kb, expT, nkb=nkb, p=p, qc=qc,
                           ctx1=ctx1, ctx2=ctx2):
                        st, sp = kb == 0, kb == nkb - 1
                        oi = kb - 4 * qc
                        off = 128 * oi if oi > 0 else 0
                        nc.tensor.matmul(
                            ctx1[:, off:512], vt[p][kb][:, 0:128],
                            expT[:, off:512],
                            start=st, stop=sp, skip_group_check=(off > 0),
                        )
                        nc.tensor.matmul(
                            ctx2[:, off:512], vt[p][kb][:, 64:192],
                            expT[:, 512 + off:1024],
                            start=st, stop=sp, skip_group_check=(off > 0),
                        )
                        if not sp:
                            return
                        # chunk close: den rows + unnormalized ctx -> bf16
                        den = attp.tile([P, 512], F32, tag="den", bufs=2,
                                        name="den")
                        nc.vector.tensor_copy(den[64:65, :], ctx1[64:65, :])
                        nc.vector.tensor_copy(den[32:33, :], ctx2[32:33, :])
                        nc.vector.tensor_copy(
                            ctxn[p][qc][0:64, :], ctx1[0:64, :]
                        )
                        nc.vector.tensor_copy(
                            ctxn[p][qc][64:128, :], ctx2[64:128, :]
                        )

                        def norm(den=den, p=p, qc=qc):
                            rec = attp.tile([P, 512], F32, tag="rec",
                                            bufs=2, name="rec")
                            for rc in range(4):
                                rsl = slice(rc * 128, (rc + 1) * 128)
                                nc.vector.reciprocal(rec[:, rsl], den[:, rsl])
                            d_t = dscp.tile([2, 512], F32, tag="d", bufs=2,
                                            name="d_t")
                            nc.sync.dma_start(d_t[0:1, :], rec[64:65, :])
                            nc.sync.dma_start(d_t[1:2, :], rec[32:33, :])
                            bc = attp.tile([P, 512], F32, tag="bc",
                                           bufs=2, name="bc")
                            nc.sync.dma_start(
                                bc[0:64, :],
                                d_t[0:1, :].to_broadcast((64, 512)),
                            )
                            nc.sync.dma_start(
                                bc[64:128, :],
                                d_t[1:2, :].to_broadcast((64, 512)),
                            )
                            nc.vector.tensor_mul(
                                ctxn[p][qc][0:64, :], ctxn[p][qc][0:64, :],
                                bc[0:64, :],
                            )
                            nc.vector.tensor_mul(
                                ctxn[p][qc][64:128, :],
                                ctxn[p][qc][64:128, :], bc[64:128, :],
                            )

                        pending.append(norm)

                    for kb in range(nkb):
                        if kb == 4:
                            flush_pending()
                            chunks_flushed[0] = 2 * qc + p - 1
                            while (oproj_pushed[0] + 1) * 2 - 1 <= chunks_flushed[0]:
                                filler.extend(outproj_closures(oproj_pushed[0]))
                                oproj_pushed[0] += 1
                        pop_filler()
                        ksl = slice(kb * P, (kb + 1) * P)
                        s_ps = psp.tile([P, 1024], F32, tag="s", bufs=2)
                        nc.tensor.matmul(
                            s_ps[:, 0:512], kT[p][0:64, ksl], qT[p][0:64, qsl],
                            start=True, stop=True, tile_position=(0, 0),
                        )
                        nc.tensor.matmul(
                            s_ps[:, 512:1024], kT[p][64:128, ksl],
                            qT[p][64:128, qsl],
                            start=True, stop=True, tile_position=(64, 0),
                        )
                        oi = kb - 4 * qc
                        off = 128 * oi
                        if oi >= 0:
                            # triangular strip mask on both halves
                            nc.vector.tensor_add(
                                s_ps[:, off:off + 128], s_ps[:, off:off + 128],
                                tri_sb[:],
                            )
                            nc.vector.tensor_add(
                                s_ps[:, 512 + off:640 + off],
                                s_ps[:, 512 + off:640 + off], tri_sb[:],
                            )
                        expT = attp.tile([P, 1024], BF16, tag="exp", bufs=6)
                        nc.scalar.activation(expT[:], s_ps[:], EXP)
                        if prev[0] is not None:
                            prev[0][0](*prev[0][1])
                        prev[0] = (av, (kb, expT))
            if prev[0] is not None:
                prev[0][0](*prev[0][1])
            flush_pending()
            while filler:
                filler.pop(0)[1]()
            while oproj_pushed[0] < NQC:
                for _, fn in outproj_closures(oproj_pushed[0]):
                    fn()
                oproj_pushed[0] += 1

    nc.compile()
    return nc


def _get_nc():
    if "nc" not in _CACHE:
        _CACHE["nc"] = _build()
    return _CACHE["nc"]


def kernel(X, W_q, W_k, W_v, W_o, b_o):
    import ml_dtypes
    from concourse.bass_utils import run_bass_kernel_spmd

    BF = ml_dtypes.bfloat16
    X = np.asarray(X, dtype=np.float32)
    # fold the 1/sqrt(head_dim) softmax scale into W_q
    W_q = (np.asarray(W_q, dtype=np.float32) * 0.125).astype(BF)
    W_k = np.asarray(W_k, dtype=np.float32).astype(BF)
    W_v = np.asarray(W_v, dtype=np.float32).astype(BF)
    W_o = np.asarray(W_o, dtype=np.float32).astype(BF)
    b_o = np.asarray(b_o, dtype=np.float32)
    Xb = X.astype(BF)

    nc = _get_nc()
    # triangular strip mask: row kp masks columns j < kp (key > query)
    kp = np.arange(128)[:, None]
    j = np.arange(128)[None, :]
    tri = np.where(kp <= j, 0.0, NEG).astype(np.float32)

    in_maps = []
    for c in range(NCORES):
        b = c // 4
        hb = c % 4
        cs = slice(hb * CLOC, (hb + 1) * CLOC)
        xt = Xb[b].T  # [1024, 2048]
        xr = np.ascontiguousarray(
            xt.reshape(NCB, 128, NQC, 512).transpose(2, 0, 1, 3)
        ).reshape(NQC * NCB * 128, 512)

        def wrearr(W):
            # [1024, 256] -> [128, 8*256] cb-major columns
            return np.ascontiguousarray(
                W.reshape(NCB, 128, CLOC).transpose(1, 0, 2)
            ).reshape(128, NCB * CLOC)

        in_maps.append({
            "Xr": xr,
            "Wq": wrearr(W_q[:, cs]),
            "Wk": wrearr(W_k[:, cs]),
            "Wv": wrearr(W_v[:, cs]),
            "Wo": np.ascontiguousarray(W_o[cs, :]),
            "tri": tri,
        })

    global _last_in_maps
    _last_in_maps = in_maps
    res = run_bass_kernel_spmd(nc, in_maps, list(range(NCORES)))
    out = np.empty((B, NTOK, DIN), dtype=np.float32)
    for b in range(B):
        acc = res.results[4 * b]["out"].astype(np.float32)
        for hb in range(1, 4):
            acc = acc + res.results[4 * b + hb]["out"]
        out[b] = acc + b_o[None, :]
    return out


# revision 17
# speedup vs baseline: 1.4660x; 1.0033x over previous
"""Multi-head causal attention on 8 Trainium2 NeuronCores.

Problem: X [2, 2048, 1024] f32, W_q/W_k/W_v [1024, 1024], W_o [1024, 1024],
b_o [1024]; 16 heads, head_dim 64, causal softmax attention + out projection.

Sharding: 2 (batch) x 4 (head-blocks of 4 heads) = 8 cores. Each core
computes q/k/v for its 4 heads on its batch, causal attention, and a partial
output projection ctx @ W_o[rows]. Host sums the 4 partials per batch and
adds b_o. No cross-core collectives.

v2 vs v1 (233us):
  - X transposed by DMA (XBAR dma_start_transpose) instead of PE matmuls.
  - Single pool scope: QKV projection work is emitted as "filler" inside the
    attention k-block loop so the Tile scheduler overlaps it with the
    ACT-bound softmax-exp stream (exp = (N+352)/1.2 ns on ScalarE is the
    phase-B floor).
  - ScalarE kept exp-only (all copies pinned to nc.vector; ones-columns via
    gpsimd memset).
  - Normalization: reciprocal_approx_fast (1 DVE op, ~5x faster than
    reciprocal) + gpsimd partition_broadcast instead of DVE reciprocal +
    DMA round-trip broadcasts; ctx drained unnormalized to bf16 and scaled
    in place.
  - PSUM: s x2 (4 banks) + ctx1 + ctx2 + qkv + oproj = 8 banks.
  - exp always one [128,1024] call per k-block (scale 0.125 folded into W_q
    host-side); AV skips fully-masked columns as before.
"""
import sys

sys.path.insert(0, "/opt/trn_rl_repo")

import numpy as np

NEG = -1.0e9
B, NTOK, DIN = 2, 2048, 1024
NH, HD = 16, 64
HPC = 4            # heads per core
CLOC = HPC * HD    # 256 local channels
NCORES = 8
NTB = NTOK // 128  # 16 token blocks
NQC = NTOK // 512  # 4 q-chunks
NCB = DIN // 128   # 8 contraction blocks

_CACHE = {}
_last_in_maps = None


def _build():
    from concourse import bacc, mybir, tile

    F32 = mybir.dt.float32
    BF16 = mybir.dt.bfloat16
    EXP = mybir.ActivationFunctionType.Exp
    P = 128

    nc = bacc.Bacc(None)
    # host-transposed X in (qc, cb)-block-contiguous layout: block (qc, cb)
    # at rows [(qc*NCB+cb)*128, +128) is XT[cb*128:(cb+1)*128, qc*512:+512]
    Xr = nc.declare_dram_parameter("Xr", [NQC * NCB * P, 512], BF16,
                                   isOutput=False)
    # weights cb-major along columns: [:, cb*256:(cb+1)*256] = W[cb block]
    Wq = nc.declare_dram_parameter("Wq", [P, NCB * CLOC], BF16, isOutput=False)
    Wk = nc.declare_dram_parameter("Wk", [P, NCB * CLOC], BF16, isOutput=False)
    Wv = nc.declare_dram_parameter("Wv", [P, NCB * CLOC], BF16, isOutput=False)
    Wo = nc.declare_dram_parameter("Wo", [CLOC, DIN], BF16, isOutput=False)
    tri = nc.declare_dram_parameter("tri", [P, P], F32, isOutput=False)
    out = nc.declare_dram_parameter("out", [NTOK, DIN], F32, isOutput=True)

    with tile.TileContext(nc) as tc:
        with (
            tc.tile_pool(name="const", bufs=1) as constp,
            tc.tile_pool(name="xt", bufs=1) as xtp,
            tc.tile_pool(name="w", bufs=1) as wp,
            tc.tile_pool(name="qkT", bufs=1) as qkTp,
            tc.tile_pool(name="vt", bufs=1) as vtp,
            tc.tile_pool(name="ctxn", bufs=1) as ctxnp,
            tc.tile_pool(name="att", bufs=1) as attp,
            tc.tile_pool(name="osb", bufs=1) as osbp,
            tc.tile_pool(name="dsc", bufs=1, space="DRAM") as dscp,
            tc.tile_pool(name="ps", bufs=1, space="PSUM") as psp,
        ):
            tri_sb = constp.tile([P, P], F32, tag="tri")
            nc.sync.dma_start(tri_sb[:], tri[:])

            # weights: one wide tile per matrix, single contiguous DMA each
            wq_sb = wp.tile([P, NCB * CLOC], BF16, tag="wq", name="wq")
            wk_sb = wp.tile([P, NCB * CLOC], BF16, tag="wk", name="wk")
            wv_sb = wp.tile([P, NCB * CLOC], BF16, tag="wv", name="wv")
            wo_sb = [wp.tile([P, DIN], BF16, tag=f"wo{p}", name=f"wo{p}")
                     for p in range(2)]
            XT = [
                [xtp.tile([P, 512], BF16, tag=f"xt{cb}_{q}", name=f"xt{cb}_{q}")
                 for q in range(NQC)]
                for cb in range(NCB)
            ]

            def load_xt(q, eng):
                for cb in range(NCB):
                    r0 = (q * NCB + cb) * P
                    eng.dma_start(XT[cb][q][:], Xr[r0:r0 + P, :])

            # ordered by first need; front-critical loads on sync, bulk on
            # the scalar queue (idle until the first exp ~16us in)
            nc.sync.dma_start(wq_sb[:], Wq[:])
            nc.scalar.dma_start(wv_sb[:], Wv[:])
            load_xt(0, nc.sync)
            nc.sync.dma_start(wk_sb[:], Wk[:])
            for p in range(2):
                nc.scalar.dma_start(wo_sb[p][:], Wo[p * P:(p + 1) * P, :])
            load_xt(1, nc.sync)
            load_xt(2, nc.scalar)
            load_xt(3, nc.scalar)

            # static result tiles
            qT = [qkTp.tile([P, NTOK], BF16, tag=f"qT{p}", name=f"qT{p}")
                  for p in range(2)]
            kT = [qkTp.tile([P, NTOK], BF16, tag=f"kT{p}", name=f"kT{p}")
                  for p in range(2)]
            # v tiles [128 keys, 192]: [vA | ones | pad | ones | pad | vB]
            vt = [
                [vtp.tile([P, 192], BF16, tag=f"vt{p}_{tb}", name=f"vt{p}_{tb}")
                 for tb in range(NTB)]
                for p in range(2)
            ]
            ctxn = [
                [ctxnp.tile([P, 512], BF16, tag=f"ctxn{p}_{qc}",
                            name=f"ctxn{p}_{qc}") for qc in range(NQC)]
                for p in range(2)
            ]
            for p in range(2):
                for tb in range(NTB):
                    t = vt[p][tb]
                    nc.gpsimd.memset(t[:, 64:128], 0.0)
                    nc.gpsimd.memset(t[:, 64:65], 1.0)
                    nc.gpsimd.memset(t[:, 96:97], 1.0)

            # ---------------- emission helpers ----------------
            # pe_work / dve_work: queues of (key, closure); each closure is
            # <=2 matmuls (or one DVE op chain link) so the in-order engine
            # queues interleave finely with the scores/exp/AV stream.
            pe_work = []
            dve_work = []
            emitted = {}   # key -> remaining closures not yet emitted

            def push_pe(key, fn):
                pe_work.append((key, fn))
                emitted[key] = emitted.get(key, 0) + 1

            def pop_pe(n=1):
                for _ in range(n):
                    if not pe_work:
                        return
                    k, fn = pe_work.pop(0)
                    fn()
                    emitted[k] -= 1

            def pop_dve():
                if dve_work:
                    dve_work.pop(0)[1]()

            def drain_until(keys):
                need = [k for k in keys if emitted.get(k, 0) > 0]
                while need:
                    k, fn = pe_work.pop(0)
                    fn()
                    emitted[k] -= 1
                    need = [k for k in keys if emitted.get(k, 0) > 0]

            def emit_qk_pair(w_sb, dst, p, qc, quarter, state):
                if quarter == 0:
                    state["t"] = psp.tile([P, 512], F32, tag="qkv", bufs=1,
                                          name="qk_ps")
                t = state["t"]
                for cb in range(2 * quarter, 2 * quarter + 2):
                    csl = slice(cb * CLOC + p * P, cb * CLOC + (p + 1) * P)
                    nc.tensor.matmul(
                        t[:], w_sb[:, csl], XT[cb][qc][:],
                        start=(cb == 0), stop=(cb == NCB - 1),
                    )
                if quarter == 3:
                    nc.vector.tensor_copy(
                        dst[p][:, qc * 512:(qc + 1) * 512], t[:]
                    )

            def emit_v_pair(tb, quarter, state):
                if quarter == 0:
                    state["t"] = psp.tile([P, 512], F32, tag="qkv", bufs=1,
                                          name="v_ps")
                t = state["t"]
                for cb in range(2 * quarter, 2 * quarter + 2):
                    nc.tensor.matmul(
                        t[:, 0:CLOC],
                        XT[cb][tb // 4][:, (tb % 4) * P:(tb % 4 + 1) * P],
                        wv_sb[:, cb * CLOC:(cb + 1) * CLOC],
                        start=(cb == 0), stop=(cb == NCB - 1),
                    )
                if quarter == 3:
                    for p in range(2):
                        hA, hB = 2 * p, 2 * p + 1
                        nc.vector.tensor_copy(
                            vt[p][tb][:, 0:64], t[:, hA * 64:(hA + 1) * 64]
                        )
                        nc.vector.tensor_copy(
                            vt[p][tb][:, 128:192], t[:, hB * 64:(hB + 1) * 64]
                        )

            def push_qk(p, qc):
                for nm, w_sb, dst in (("q", wq_sb, qT), ("k", wk_sb, kT)):
                    st = {}
                    for quarter in range(4):
                        push_pe(
                            (nm, p, qc),
                            lambda w_sb=w_sb, dst=dst, p=p, qc=qc,
                            quarter=quarter, st=st:
                            emit_qk_pair(w_sb, dst, p, qc, quarter, st)
                        )

            def push_v(tb):
                st = {}
                for quarter in range(4):
                    push_pe(
                        ("v", tb),
                        lambda tb=tb, quarter=quarter, st=st:
                        emit_v_pair(tb, quarter, st)
                    )

            def emit_outproj_tile(qc, i, jc):
                tb = 4 * qc + i
                tsl = slice(tb * P, (tb + 1) * P)
                jsl = slice(jc * 512, (jc + 1) * 512)
                o_ps = psp.tile([P, 512], F32, tag="oproj", bufs=1,
                                name="o_ps")
                for pp in range(2):
                    nc.tensor.matmul(
                        o_ps[:],
                        ctxn[pp][qc][:, i * P:(i + 1) * P],
                        wo_sb[pp][:, jsl],
                        start=(pp == 0), stop=(pp == 1),
                    )
                o_sb = osbp.tile([P, 512], F32, tag="o_sb", bufs=4,
                                 name="o_sb")
                nc.vector.tensor_copy(o_sb[:], o_ps[:])
                nc.sync.dma_start(out[tsl, jsl], o_sb[:])

            # norm bookkeeping: when both chunks of a qc have their norm
            # fully emitted, its outproj tiles become pe filler
            norm_done = [0, 0, 0, 0]

            def norm_complete(qc):
                norm_done[qc] += 1
                if norm_done[qc] == 2:
                    for i in range(4):
                        for jc in range(2):
                            push_pe(
                                ("o", qc, i, jc),
                                lambda qc=qc, i=i, jc=jc:
                                emit_outproj_tile(qc, i, jc)
                            )

            def push_norm(den, p, qc):
                rec = attp.tile([P, 512], F32, tag="rec", bufs=2, name="rec")
                bc = attp.tile([P, 512], F32, tag="bc", bufs=2, name="bc")
                d_t = dscp.tile([2, 512], F32, tag="d", bufs=2, name="d_t")

                def recq(rc, rec=rec, den=den):
                    rsl = slice(rc * 128, (rc + 1) * 128)
                    nc.vector.reciprocal(rec[:, rsl], den[:, rsl])

                def dmas(rec=rec, d_t=d_t, bc=bc):
                    nc.sync.dma_start(d_t[0:1, :], rec[64:65, :])
                    nc.sync.dma_start(d_t[1:2, :], rec[32:33, :])
                    nc.sync.dma_start(
                        bc[0:64, :], d_t[0:1, :].to_broadcast((64, 512))
                    )
                    nc.sync.dma_start(
                        bc[64:128, :], d_t[1:2, :].to_broadcast((64, 512))
                    )

                def muls(bc=bc, p=p, qc=qc):
                    nc.vector.tensor_mul(
                        ctxn[p][qc][0:64, :], ctxn[p][qc][0:64, :],
                        bc[0:64, :],
                    )
                    nc.vector.tensor_mul(
                        ctxn[p][qc][64:128, :],
                        ctxn[p][qc][64:128, :], bc[64:128, :],
                    )
                    norm_complete(qc)

                for rc in range(4):
                    dve_work.append(((p, qc), lambda rc=rc: recq(rc)))
                dve_work.append(((p, qc), dmas))
                dve_work.append(((p, qc), muls))

            # ---------------- attention ----------------
            # preload q/k for chunk (0,0) directly; everything else queued
            st = {}
            for quarter in range(4):
                emit_qk_pair(wq_sb, qT, 0, 0, quarter, st)
            st = {}
            for quarter in range(4):
                emit_qk_pair(wk_sb, kT, 0, 0, quarter, st)
            for tb in range(4):
                push_v(tb)
            push_qk(1, 0)

            prev = [None]          # pipelined AV across kb boundaries

            for qc in range(NQC):
                if qc + 1 < NQC:
                    for tb in range(4 * qc + 4, 4 * qc + 8):
                        push_v(tb)
                    push_qk(0, qc + 1)
                    push_qk(1, qc + 1)
                for p in range(2):
                    drain_until([("q", p, qc), ("k", p, qc)])
                    qsl = slice(qc * 512, (qc + 1) * 512)
                    nkb = 4 * qc + 4
                    ctx1 = psp.tile([P, 512], F32, tag="ctx1", bufs=1)
                    ctx2 = psp.tile([P, 512], F32, tag="ctx2", bufs=1)

                    def av(kb, expT, nkb=nkb, p=p, qc=qc,
                           ctx1=ctx1, ctx2=ctx2):
                        st_, sp = kb == 0, kb == nkb - 1
                        oi = kb - 4 * qc
                        off = 128 * oi if oi > 0 else 0
                        nc.tensor.matmul(
                            ctx1[:, off:512], vt[p][kb][:, 0:128],
                            expT[:, off:512],
                            start=st_, stop=sp, skip_group_check=(off > 0),
                        )
                        nc.tensor.matmul(
                            ctx2[:, off:512], vt[p][kb][:, 64:192],
                            expT[:, 512 + off:1024],
                            start=st_, stop=sp, skip_group_check=(off > 0),
                        )
                        if not sp:
                            return
                        # chunk close: den rows + unnormalized ctx -> bf16
                        den = attp.tile([P, 512], F32, tag="den", bufs=2,
                                        name="den")
                        nc.vector.tensor_copy(den[64:65, :], ctx1[64:65, :])
                        nc.vector.tensor_copy(den[32:33, :], ctx2[32:33, :])
                        nc.vector.tensor_copy(
                            ctxn[p][qc][0:64, :], ctx1[0:64, :]
                        )
                        nc.vector.tensor_copy(
                            ctxn[p][qc][64:128, :], ctx2[64:128, :]
                        )
                        push_norm(den, p, qc)

                    for kb in range(nkb):
                        ksl = slice(kb * P, (kb + 1) * P)
                        s_ps = psp.tile([P, 1024], F32, tag="s", bufs=2)
                        nc.tensor.matmul(
                            s_ps[:, 0:512], kT[p][0:64, ksl], qT[p][0:64, qsl],
                            start=True, stop=True, tile_position=(0, 0),
                        )
                        nc.tensor.matmul(
                            s_ps[:, 512:1024], kT[p][64:128, ksl],
                            qT[p][64:128, qsl],
                            start=True, stop=True, tile_position=(64, 0),
                        )
                        oi = kb - 4 * qc
                        off = 128 * oi
                        if oi >= 0:
                            # triangular strip mask on both halves
                            nc.vector.tensor_add(
                                s_ps[:, off:off + 128], s_ps[:, off:off + 128],
                                tri_sb[:],
                            )
                            nc.vector.tensor_add(
                                s_ps[:, 512 + off:640 + off],
                                s_ps[:, 512 + off:640 + off], tri_sb[:],
                            )
                        expT = attp.tile([P, 1024], BF16, tag="exp", bufs=6)
                        nc.scalar.activation(expT[:], s_ps[:], EXP)
                        drain_until([("v", kb)])
                        if prev[0] is not None:
                            prev[0][0](*prev[0][1])
                        prev[0] = (av, (kb, expT))
                        pop_pe(2 if len(pe_work) > 16 else 1)
                        pop_dve()
            if prev[0] is not None:
                drain_until([("v", NTB - 1)])
                prev[0][0](*prev[0][1])
            while dve_work:
                pop_dve()
            while pe_work:
                pop_pe()

    nc.compile()
    return nc


def _get_nc():
    if "nc" not in _CACHE:
        _CACHE["nc"] = _build()
    return _CACHE["nc"]


def kernel(X, W_q, W_k, W_v, W_o, b_o):
    import ml_dtypes
    from concourse.bass_utils import run_bass_kernel_spmd

    BF = ml_dtypes.bfloat16
    X = np.asarray(X, dtype=np.float32)
    # fold the 1/sqrt(head_dim) softmax scale into W_q
    W_q = (np.asarray(W_q, dtype=np.float32) * 0.125).astype(BF)
    W_k = np.asarray(W_k, dtype=np.float32).astype(BF)
    W_v = np.asarray(W_v, dtype=np.float32).astype(BF)
    W_o = np.asarray(W_o, dtype=np.float32).astype(BF)
    b_o = np.asarray(b_o, dtype=np.float32)
    Xb = X.astype(BF)

    nc = _get_nc()
    # triangular strip mask: row kp masks columns j < kp (key > query)
    kp = np.arange(128)[:, None]
    j = np.arange(128)[None, :]
    tri = np.where(kp <= j, 0.0, NEG).astype(np.float32)

    in_maps = []
    for c in range(NCORES):
        b = c // 4
        hb = c % 4
        cs = slice(hb * CLOC, (hb + 1) * CLOC)
        xt = Xb[b].T  # [1024, 2048]
        xr = np.ascontiguousarray(
            xt.reshape(NCB, 128, NQC, 512).transpose(2, 0, 1, 3)
        ).reshape(NQC * NCB * 128, 512)

        def wrearr(W):
            # [1024, 256] -> [128, 8*256] cb-major columns
            return np.ascontiguousarray(
                W.reshape(NCB, 128, CLOC).transpose(1, 0, 2)
            ).reshape(128, NCB * CLOC)

        in_maps.append({
            "Xr": xr,
            "Wq": wrearr(W_q[:, cs]),
            "Wk": wrearr(W_k[:, cs]),
            "Wv": wrearr(W_v[:, cs]),
            "Wo": np.ascontiguousarray(W_o[cs, :]),
            "tri": tri,
        })

    global _last_in_maps
    _last_in_maps = in_maps
    res = run_bass_kernel_spmd(nc, in_maps, list(range(NCORES)))
    out = np.empty((B, NTOK, DIN), dtype=np.float32)
    for b in range(B):
        acc = res.results[4 * b]["out"].astype(np.float32)
        for hb in range(1, 4):
            acc = acc + res.results[4 * b + hb]["out"]
        out[b] = acc + b_o[None, :]
    return out


# revision 18
# speedup vs baseline: 1.6110x; 1.0989x over previous
"""Multi-head causal attention on 8 Trainium2 NeuronCores.

Problem: X [2, 2048, 1024] f32, W_q/W_k/W_v [1024, 1024], W_o [1024, 1024],
b_o [1024]; 16 heads, head_dim 64, causal softmax attention + out projection.

Sharding: 2 (batch) x 4 (head-blocks of 4 heads) = 8 cores. Each core
computes q/k/v for its 4 heads on its batch, causal attention, and a partial
output projection ctx @ W_o[rows]. Host sums the 4 partials per batch and
adds b_o. No cross-core collectives.

v2 vs v1 (233us):
  - X transposed by DMA (XBAR dma_start_transpose) instead of PE matmuls.
  - Single pool scope: QKV projection work is emitted as "filler" inside the
    attention k-block loop so the Tile scheduler overlaps it with the
    ACT-bound softmax-exp stream (exp = (N+352)/1.2 ns on ScalarE is the
    phase-B floor).
  - ScalarE kept exp-only (all copies pinned to nc.vector; ones-columns via
    gpsimd memset).
  - Normalization: reciprocal_approx_fast (1 DVE op, ~5x faster than
    reciprocal) + gpsimd partition_broadcast instead of DVE reciprocal +
    DMA round-trip broadcasts; ctx drained unnormalized to bf16 and scaled
    in place.
  - PSUM: s x2 (4 banks) + ctx1 + ctx2 + qkv + oproj = 8 banks.
  - exp always one [128,1024] call per k-block (scale 0.125 folded into W_q
    host-side); AV skips fully-masked columns as before.
"""
import sys

sys.path.insert(0, "/opt/trn_rl_repo")

import numpy as np

NEG = -1.0e9
B, NTOK, DIN = 2, 2048, 1024
NH, HD = 16, 64
HPC = 4            # heads per core
CLOC = HPC * HD    # 256 local channels
NCORES = 8
NTB = NTOK // 128  # 16 token blocks
NQC = NTOK // 512  # 4 q-chunks
NCB = DIN // 128   # 8 contraction blocks

_CACHE = {}
_last_in_maps = None


def _build():
    from concourse import bacc, mybir, tile

    F32 = mybir.dt.float32
    BF16 = mybir.dt.bfloat16
    EXP = mybir.ActivationFunctionType.Exp
    P = 128

    nc = bacc.Bacc(None)
    # host-transposed X in (qc, cb)-block-contiguous layout: block (qc, cb)
    # at rows [(qc*NCB+cb)*128, +128) is XT[cb*128:(cb+1)*128, qc*512:+512]
    Xr = nc.declare_dram_parameter("Xr", [NQC * NCB * P, 512], BF16,
                                   isOutput=False)
    # weights cb-major along columns: [:, cb*256:(cb+1)*256] = W[cb block]
    Wq = nc.declare_dram_parameter("Wq", [P, NCB * CLOC], BF16, isOutput=False)
    Wk = nc.declare_dram_parameter("Wk", [P, NCB * CLOC], BF16, isOutput=False)
    Wv = nc.declare_dram_parameter("Wv", [P, NCB * CLOC], BF16, isOutput=False)
    Wo = nc.declare_dram_parameter("Wo", [CLOC, DIN], BF16, isOutput=False)
    tri = nc.declare_dram_parameter("tri", [P, P], F32, isOutput=False)
    out = nc.declare_dram_parameter("out", [NTOK, DIN], F32, isOutput=True)

    with tile.TileContext(nc) as tc:
        with (
            tc.tile_pool(name="const", bufs=1) as constp,
            tc.tile_pool(name="xt", bufs=1) as xtp,
            tc.tile_pool(name="w", bufs=1) as wp,
            tc.tile_pool(name="qkT", bufs=1) as qkTp,
            tc.tile_pool(name="vt", bufs=1) as vtp,
            tc.tile_pool(name="ctxn", bufs=1) as ctxnp,
            tc.tile_pool(name="att", bufs=1) as attp,
            tc.tile_pool(name="osb", bufs=1) as osbp,
            tc.tile_pool(name="dsc", bufs=1, space="DRAM") as dscp,
            tc.tile_pool(name="ps", bufs=1, space="PSUM") as psp,
        ):
            tri_sb = constp.tile([P, P], F32, tag="tri")
            nc.sync.dma_start(tri_sb[:], tri[:])

            # weights: one wide tile per matrix, single contiguous DMA each
            wq_sb = wp.tile([P, NCB * CLOC], BF16, tag="wq", name="wq")
            wk_sb = wp.tile([P, NCB * CLOC], BF16, tag="wk", name="wk")
            wv_sb = wp.tile([P, NCB * CLOC], BF16, tag="wv", name="wv")
            wo_sb = [wp.tile([P, DIN], BF16, tag=f"wo{p}", name=f"wo{p}")
                     for p in range(2)]
            XT = [
                [xtp.tile([P, 512], BF16, tag=f"xt{cb}_{q}", name=f"xt{cb}_{q}")
                 for q in range(NQC)]
                for cb in range(NCB)
            ]

            def load_xt(q, eng):
                for cb in range(NCB):
                    r0 = (q * NCB + cb) * P
                    eng.dma_start(XT[cb][q][:], Xr[r0:r0 + P, :])

            # ordered by first need; front-critical loads on sync, bulk on
            # the scalar queue (idle until the first exp ~16us in)
            nc.sync.dma_start(wq_sb[:], Wq[:])
            nc.scalar.dma_start(wv_sb[:], Wv[:])
            load_xt(0, nc.sync)
            nc.sync.dma_start(wk_sb[:], Wk[:])
            for p in range(2):
                nc.scalar.dma_start(wo_sb[p][:], Wo[p * P:(p + 1) * P, :])
            load_xt(1, nc.sync)
            load_xt(2, nc.scalar)
            load_xt(3, nc.scalar)

            # static result tiles
            qT = [qkTp.tile([P, NTOK], BF16, tag=f"qT{p}", name=f"qT{p}")
                  for p in range(2)]
            kT = [qkTp.tile([P, NTOK], BF16, tag=f"kT{p}", name=f"kT{p}")
                  for p in range(2)]
            # v tiles [128 keys, 192]: [vA | ones | pad | ones | pad | vB]
            vt = [
                [vtp.tile([P, 192], BF16, tag=f"vt{p}_{tb}", name=f"vt{p}_{tb}")
                 for tb in range(NTB)]
                for p in range(2)
            ]
            ctxn = [
                [ctxnp.tile([P, 512], BF16, tag=f"ctxn{p}_{qc}",
                            name=f"ctxn{p}_{qc}") for qc in range(NQC)]
                for p in range(2)
            ]
            for p in range(2):
                for tb in range(NTB):
                    t = vt[p][tb]
                    nc.gpsimd.memset(t[:, 64:128], 0.0)
                    nc.gpsimd.memset(t[:, 64:65], 1.0)
                    nc.gpsimd.memset(t[:, 96:97], 1.0)

            # ---------------- emission helpers ----------------
            # pe_work / dve_work: queues of (key, closure); each closure is
            # <=2 matmuls (or one DVE op chain link) so the in-order engine
            # queues interleave finely with the scores/exp/AV stream.
            pe_work = []
            dve_work = []
            emitted = {}   # key -> remaining closures not yet emitted

            def push_pe(key, fn):
                pe_work.append((key, fn))
                emitted[key] = emitted.get(key, 0) + 1

            def pop_pe(n=1):
                for _ in range(n):
                    if not pe_work:
                        return
                    k, fn = pe_work.pop(0)
                    fn()
                    emitted[k] -= 1

            def pop_dve():
                if dve_work:
                    dve_work.pop(0)[1]()

            def drain_until(keys):
                need = [k for k in keys if emitted.get(k, 0) > 0]
                while need:
                    k, fn = pe_work.pop(0)
                    fn()
                    emitted[k] -= 1
                    need = [k for k in keys if emitted.get(k, 0) > 0]

            def emit_qk_pair(w_sb, dst, p, qc, quarter, state):
                if quarter == 0:
                    state["t"] = psp.tile([P, 512], F32, tag="fill", bufs=2,
                                          name="qk_ps")
                t = state["t"]
                for cb in range(2 * quarter, 2 * quarter + 2):
                    csl = slice(cb * CLOC + p * P, cb * CLOC + (p + 1) * P)
                    nc.tensor.matmul(
                        t[:], w_sb[:, csl], XT[cb][qc][:],
                        start=(cb == 0), stop=(cb == NCB - 1),
                    )
                if quarter == 3:
                    nc.vector.tensor_copy(
                        dst[p][:, qc * 512:(qc + 1) * 512], t[:]
                    )

            def emit_v_pair(tb, quarter, state):
                if quarter == 0:
                    state["t"] = psp.tile([P, 512], F32, tag="fill", bufs=2,
                                          name="v_ps")
                t = state["t"]
                for cb in range(2 * quarter, 2 * quarter + 2):
                    nc.tensor.matmul(
                        t[:, 0:CLOC],
                        XT[cb][tb // 4][:, (tb % 4) * P:(tb % 4 + 1) * P],
                        wv_sb[:, cb * CLOC:(cb + 1) * CLOC],
                        start=(cb == 0), stop=(cb == NCB - 1),
                    )
                if quarter == 3:
                    for p in range(2):
                        hA, hB = 2 * p, 2 * p + 1
                        nc.vector.tensor_copy(
                            vt[p][tb][:, 0:64], t[:, hA * 64:(hA + 1) * 64]
                        )
                        nc.vector.tensor_copy(
                            vt[p][tb][:, 128:192], t[:, hB * 64:(hB + 1) * 64]
                        )

            def push_qk(p, qc):
                for nm, w_sb, dst in (("q", wq_sb, qT), ("k", wk_sb, kT)):
                    st = {}
                    for quarter in range(4):
                        push_pe(
                            (nm, p, qc),
                            lambda w_sb=w_sb, dst=dst, p=p, qc=qc,
                            quarter=quarter, st=st:
                            emit_qk_pair(w_sb, dst, p, qc, quarter, st)
                        )

            def push_v(tb):
                st = {}
                for quarter in range(4):
                    push_pe(
                        ("v", tb),
                        lambda tb=tb, quarter=quarter, st=st:
                        emit_v_pair(tb, quarter, st)
                    )

            def emit_outproj_tile(qc, i, jc):
                tb = 4 * qc + i
                tsl = slice(tb * P, (tb + 1) * P)
                jsl = slice(jc * 512, (jc + 1) * 512)
                o_ps = psp.tile([P, 512], F32, tag="fill", bufs=2,
                                name="o_ps")
                for pp in range(2):
                    nc.tensor.matmul(
                        o_ps[:],
                        ctxn[pp][qc][:, i * P:(i + 1) * P],
                        wo_sb[pp][:, jsl],
                        start=(pp == 0), stop=(pp == 1),
                    )
                o_sb = osbp.tile([P, 512], F32, tag="o_sb", bufs=4,
                                 name="o_sb")
                if jc == 0:
                    nc.vector.tensor_copy(o_sb[:], o_ps[:])
                else:
                    nc.scalar.activation(
                        o_sb[:], o_ps[:], mybir.ActivationFunctionType.Copy
                    )
                (nc.sync if (i + jc) % 2 == 0 else nc.scalar).dma_start(
                    out[tsl, jsl], o_sb[:]
                )

            # norm bookkeeping: when both chunks of a qc have their norm
            # fully emitted, its outproj tiles become pe filler
            norm_done = [0, 0, 0, 0]

            def norm_complete(qc):
                norm_done[qc] += 1
                if norm_done[qc] == 2:
                    for i in range(4):
                        for jc in range(2):
                            push_pe(
                                ("o", qc, i, jc),
                                lambda qc=qc, i=i, jc=jc:
                                emit_outproj_tile(qc, i, jc)
                            )

            def push_norm(den, p, qc):
                rec = attp.tile([P, 512], F32, tag="rec", bufs=2, name="rec")
                bc = attp.tile([P, 512], F32, tag="bc", bufs=2, name="bc")
                d_t = dscp.tile([2, 512], F32, tag="d", bufs=2, name="d_t")

                def recq(rc, rec=rec, den=den):
                    rsl = slice(rc * 128, (rc + 1) * 128)
                    nc.vector.reciprocal(rec[:, rsl], den[:, rsl])

                def dmas(rec=rec, d_t=d_t, bc=bc):
                    nc.sync.dma_start(d_t[0:1, :], rec[64:65, :])
                    nc.sync.dma_start(d_t[1:2, :], rec[32:33, :])
                    nc.sync.dma_start(
                        bc[0:64, :], d_t[0:1, :].to_broadcast((64, 512))
                    )
                    nc.sync.dma_start(
                        bc[64:128, :], d_t[1:2, :].to_broadcast((64, 512))
                    )

                def muls(bc=bc, p=p, qc=qc):
                    nc.vector.tensor_mul(
                        ctxn[p][qc][0:64, :], ctxn[p][qc][0:64, :],
                        bc[0:64, :],
                    )
                    nc.vector.tensor_mul(
                        ctxn[p][qc][64:128, :],
                        ctxn[p][qc][64:128, :], bc[64:128, :],
                    )
                    norm_complete(qc)

                for rc in range(4):
                    dve_work.append(((p, qc), lambda rc=rc: recq(rc)))
                dve_work.append(((p, qc), dmas))
                dve_work.append(((p, qc), muls))

            # ---------------- attention ----------------
            # preload q/k for chunk (0,0) directly; everything else queued
            st = {}
            for quarter in range(4):
                emit_qk_pair(wq_sb, qT, 0, 0, quarter, st)
            st = {}
            for quarter in range(4):
                emit_qk_pair(wk_sb, kT, 0, 0, quarter, st)
            for tb in range(4):
                push_v(tb)
            push_qk(1, 0)

            prev = [None]          # pipelined AV across kb boundaries

            for qc in range(NQC):
                if qc + 1 < NQC:
                    for tb in range(4 * qc + 4, 4 * qc + 8):
                        push_v(tb)
                    push_qk(0, qc + 1)
                    push_qk(1, qc + 1)
                for p in range(2):
                    drain_until([("q", p, qc), ("k", p, qc)])
                    qsl = slice(qc * 512, (qc + 1) * 512)
                    nkb = 4 * qc + 4
                    ctx1 = psp.tile([P, 512], F32, tag="ctx1", bufs=1)
                    ctx2 = psp.tile([P, 512], F32, tag="ctx2", bufs=1)

                    def av(kb, expT, nkb=nkb, p=p, qc=qc,
                           ctx1=ctx1, ctx2=ctx2):
                        st_, sp = kb == 0, kb == nkb - 1
                        oi = kb - 4 * qc
                        off = 128 * oi if oi > 0 else 0
                        nc.tensor.matmul(
                            ctx1[:, off:512], vt[p][kb][:, 0:128],
                            expT[:, off:512],
                            start=st_, stop=sp, skip_group_check=(off > 0),
                        )
                        nc.tensor.matmul(
                            ctx2[:, off:512], vt[p][kb][:, 64:192],
                            expT[:, 512 + off:1024],
                            start=st_, stop=sp, skip_group_check=(off > 0),
                        )
                        if not sp:
                            return
                        # chunk close: den rows + unnormalized ctx -> bf16
                        den = attp.tile([P, 512], F32, tag="den", bufs=2,
                                        name="den")
                        nc.vector.tensor_copy(den[64:65, :], ctx1[64:65, :])
                        nc.vector.tensor_copy(den[32:33, :], ctx2[32:33, :])
                        nc.vector.tensor_copy(
                            ctxn[p][qc][0:64, :], ctx1[0:64, :]
                        )
                        nc.vector.tensor_copy(
                            ctxn[p][qc][64:128, :], ctx2[64:128, :]
                        )
                        push_norm(den, p, qc)

                    for kb in range(nkb):
                        ksl = slice(kb * P, (kb + 1) * P)
                        s_ps = psp.tile([P, 1024], F32, tag="s", bufs=2)
                        nc.tensor.matmul(
                            s_ps[:, 0:512], kT[p][0:64, ksl], qT[p][0:64, qsl],
                            start=True, stop=True, tile_position=(0, 0),
                        )
                        nc.tensor.matmul(
                            s_ps[:, 512:1024], kT[p][64:128, ksl],
                            qT[p][64:128, qsl],
                            start=True, stop=True, tile_position=(64, 0),
                        )
                        oi = kb - 4 * qc
                        off = 128 * oi
                        if oi >= 0:
                            # triangular strip mask on both halves
                            nc.vector.tensor_add(
                                s_ps[:, off:off + 128], s_ps[:, off:off + 128],
                                tri_sb[:],
                            )
                            nc.vector.tensor_add(
                                s_ps[:, 512 + off:640 + off],
                                s_ps[:, 512 + off:640 + off], tri_sb[:],
                            )
                        expT = attp.tile([P, 1024], BF16, tag="exp", bufs=6)
                        nc.scalar.activation(expT[:], s_ps[:], EXP)
                        drain_until([("v", kb)])
                        if prev[0] is not None:
                            prev[0][0](*prev[0][1])
                        prev[0] = (av, (kb, expT))
                        pop_pe(2 if len(pe_work) > 16 else 1)
                        pop_dve()
                        if len(dve_work) > 6:
                            pop_dve()
            if prev[0] is not None:
                drain_until([("v", NTB - 1)])
                prev[0][0](*prev[0][1])
            while dve_work:
                pop_dve()
            while pe_work:
                pop_pe()

    nc.compile()
    return nc


def _get_nc():
    if "nc" not in _CACHE:
        _CACHE["nc"] = _build()
    return _CACHE["nc"]


def kernel(X, W_q, W_k, W_v, W_o, b_o):
    import ml_dtypes
    from concourse.bass_utils import run_bass_kernel_spmd

    BF = ml_dtypes.bfloat16
    X = np.asarray(X, dtype=np.float32)
    # fold the 1/sqrt(head_dim) softmax scale into W_q
    W_q = (np.asarray(W_q, dtype=np.float32) * 0.125).astype(BF)
    W_k = np.asarray(W_k, dtype=np.float32).astype(BF)
    W_v = np.asarray(W_v, dtype=np.float32).astype(BF)
    W_o = np.asarray(W_o, dtype=np.float32).astype(BF)
    b_o = np.asarray(b_o, dtype=np.float32)
    Xb = X.astype(BF)

    nc = _get_nc()
    # triangular strip mask: row kp masks columns j < kp (key > query)
    kp = np.arange(128)[:, None]
    j = np.arange(128)[None, :]
    tri = np.where(kp <= j, 0.0, NEG).astype(np.float32)

    in_maps = []
    for c in range(NCORES):
        b = c // 4
        hb = c % 4
        cs = slice(hb * CLOC, (hb + 1) * CLOC)
        xt = Xb[b].T  # [1024, 2048]
        xr = np.ascontiguousarray(
            xt.reshape(NCB, 128, NQC, 512).transpose(2, 0, 1, 3)
        ).reshape(NQC * NCB * 128, 512)

        def wrearr(W):
            # [1024, 256] -> [128, 8*256] cb-major columns
            return np.ascontiguousarray(
                W.reshape(NCB, 128, CLOC).transpose(1, 0, 2)
            ).reshape(128, NCB * CLOC)

        in_maps.append({
            "Xr": xr,
            "Wq": wrearr(W_q[:, cs]),
            "Wk": wrearr(W_k[:, cs]),
            "Wv": wrearr(W_v[:, cs]),
            "Wo": np.ascontiguousarray(W_o[cs, :]),
            "tri": tri,
        })

    global _last_in_maps
    _last_in_maps = in_maps
    res = run_bass_kernel_spmd(nc, in_maps, list(range(NCORES)))
    out = np.empty((B, NTOK, DIN), dtype=np.float32)
    for b in range(B):
        acc = res.results[4 * b]["out"].astype(np.float32)
        for hb in range(1, 4):
            acc = acc + res.results[4 * b + hb]["out"]
        out[b] = acc + b_o[None, :]
    return out
